# revision 43
# baseline (speedup 1.0000x reference)
"""AdaptiveGridMerger Trainium2 kernel.

Math: the reference scatters x[b,c,:] into a flat 8x8 grid with bilinear
(4-corner) weights from positions[b,c,:], then matmuls grid_weights
(GW [270,64]). Equivalent form used here, per batch:
  S in R[64,306] holds channel c's 4 corner weights in column c
  (row c of S.T factorizes as wy (x) wx, the bilinear hat functions),
  out[0:256]  = GW[0:256] @ (S @ x)          (two matmul passes)
  out[256:270]= (GW[256:270] @ S) @ x        (tail folded into pass 1:
     Wtail.T [306,14] is appended to S.T as extra columns, so the 14
     leftover output rows cost zero extra PE streams).
All contractions run on the TensorEngine in bf16.

Sharding: data-parallel over batch, 2 batches per core, grid_weights
replicated (pre-transposed on host to [64, 270] for the lhsT layout).

Perf structure: spin matmuls pre-ramp the PE clock (HAM gate) during
setup; weights-outer matmul ordering; x tiles fully prefetched up front
on the HW-DGE path (first tile first) so output DMAs can't head-of-line
block them; PSUM->SBUF copies are split across DVE and ACT.
"""

import numpy as np

import concourse.bass as bass
import concourse.bacc as bacc
import concourse.mybir as mybir
from concourse import tile
from concourse.bass_utils import run_bass_kernel_spmd

B, C, T = 16, 306, 4096
M, G, GS = 270, 64, 8
N_CORES = 8
BL = B // N_CORES  # batches per core

C_CHUNKS = [(0, 128), (128, 128), (256, 50)]
M_CHUNKS = [(0, 128), (128, 128), (256, 14)]
M_TAIL0, M_TAIL = 256, 14
GE = G
T_DMA = 2048
T_PS = 512
NPS = T_DMA // T_PS
N_SPIN = 13

MM_DTYPE = mybir.dt.bfloat16
NP_MM = mybir.dt.np(MM_DTYPE)

FP32 = mybir.dt.float32
OP = mybir.AluOpType


def _pos_col(b, ci):
    return 2 * (b * 2 + ci) if ci < 2 else 8 + 2 * b


def build_nc():
    nc = bacc.Bacc()
    x_ext = nc.declare_dram_parameter("x", [BL, C, T], MM_DTYPE, isOutput=False)
    pos_ext = nc.declare_dram_parameter("positions", [BL, C, 2], FP32, isOutput=False)
    gwt_ext = nc.declare_dram_parameter("gw_t", [G, M], MM_DTYPE, isOutput=False)
    out_ext = nc.declare_dram_parameter("out", [BL, M, T], MM_DTYPE, isOutput=True)

    n_chunks = len(C_CHUNKS)
    with tile.TileContext(nc) as tc:
        with (
            tc.tile_pool(name="const", bufs=1) as constp,
            tc.tile_pool(name="stp", bufs=1) as stp,
            tc.tile_pool(name="scr", bufs=1) as scr,
            tc.tile_pool(name="xp", bufs=1) as xp,
            tc.tile_pool(name="op", bufs=2) as outp,
            tc.tile_pool(name="gvp", bufs=3) as gvp,
            tc.tile_pool(name="ps", bufs=4, space=bass.MemorySpace.PSUM) as psp,
        ):
            # ---- PE clock pre-ramp while DMAs/DVE setup run.
            dummy = constp.tile([128, T_PS], MM_DTYPE, tag="dummy")
            nc.gpsimd.memset(dummy[:], 0.0)
            spin_ps = psp.tile([128, 2 * T_PS], FP32, tag="pb", name="spin_ps")
            for s in range(N_SPIN):
                nc.tensor.matmul(
                    spin_ps[:, :T_PS], dummy[:, :128], dummy[:], start=True, stop=True
                )

            # t-tile 0's x chunks first: they gate the first real matmuls
            x_tiles = {}
            for ci, (c0, cn) in enumerate(C_CHUNKS):
                xt = xp.tile(
                    [128, T_DMA], MM_DTYPE, tag=f"x0_0_{ci}", name=f"x0_0_{ci}"
                )
                nc.sync.dma_start(out=xt[:cn], in_=x_ext[0, c0 : c0 + cn, 0:T_DMA])
                x_tiles[(0, 0, ci)] = xt

            for ci, (c0, cn) in enumerate(C_CHUNKS):
                xt = xp.tile(
                    [128, T_DMA], MM_DTYPE, tag=f"x0_1_{ci}", name=f"x0_1_{ci}"
                )
                nc.sync.dma_start(
                    out=xt[:cn], in_=x_ext[0, c0 : c0 + cn, T_DMA : 2 * T_DMA]
                )
                x_tiles[(0, 1, ci)] = xt

            gw_t = constp.tile([G, M], MM_DTYPE, tag="gw_t")
            nc.sync.dma_start(out=gw_t[:], in_=gwt_ext[:])

            # iota row [0..7] (cell centers)
            io_g = constp.tile([128, GS], FP32, tag="io_g")
            nc.gpsimd.iota(
                io_g[:],
                pattern=[[1, GS]],
                base=0,
                channel_multiplier=0,
                allow_small_or_imprecise_dtypes=True,
            )
            io = constp.tile([128, GS], FP32, tag="io")
            nc.vector.tensor_copy(io[:], io_g[:])

            # ---- pos -> gp for all 6 (batch, chunk) column-pairs at once
            NCOL = 2 * BL * n_chunks
            pos_all = scr.tile([128, NCOL], FP32, tag="pos_all")
            nc.vector.memset(pos_all[:], 0.0)
            for b in range(BL):
                nc.sync.dma_start(
                    out=pos_all[:, 4 * b : 4 * b + 4].rearrange(
                        "p (ci d) -> p ci d", ci=2
                    ),
                    in_=pos_ext[b, 0:256, :].rearrange("(ci p) d -> p ci d", p=128),
                )
                nc.sync.dma_start(
                    out=pos_all[:50, 8 + 2 * b : 10 + 2 * b],
                    in_=pos_ext[b, 256:306, :],
                )

            # prefetch the rest of batch 0's x tiles (batch 1's are issued
            # after the ST build so these get the full bandwidth first)
            def _prefetch_x(b, tt):
                t0 = tt * T_DMA
                for ci, (c0, cn) in enumerate(C_CHUNKS):
                    xt = xp.tile(
                        [128, T_DMA], MM_DTYPE,
                        tag=f"x{b}_{tt}_{ci}", name=f"x{b}_{tt}_{ci}",
                    )
                    nc.sync.dma_start(
                        out=xt[:cn], in_=x_ext[b, c0 : c0 + cn, t0 : t0 + T_DMA]
                    )
                    x_tiles[(b, tt, ci)] = xt



            gp = scr.tile([128, NCOL], FP32, tag="gp")
            nc.vector.tensor_scalar(gp[:], pos_all[:], 1.0, GS / 2.0, OP.add, OP.mult)
            ngp = scr.tile([128, NCOL], FP32, tag="ngp")
            nc.vector.tensor_scalar_mul(ngp[:], gp[:], -1.0)

            # ---- ST build: st[:, :64] = wy (x) wx (hat functions)
            st_tiles = {}
            for b in range(BL):
                for ci, (c0, cn) in enumerate(C_CHUNKS):
                    sfx = f"{b}_{ci}"
                    col = _pos_col(b, ci)
                    wyx = []
                    for d, nm in ((0, "wy"), (1, "wx")):
                        cd = col + d
                        # hat on ACT: u = |io - gp|, w = relu(1 - u); frees DVE
                        u = scr.tile([128, GS], FP32, tag=f"{nm}u{sfx}", name=f"{nm}u{sfx}")
                        nc.scalar.activation(
                            u[:cn], io[:cn], mybir.ActivationFunctionType.Abs,
                            bias=ngp[:cn, cd : cd + 1], scale=1.0,
                        )
                        w = scr.tile([128, GS], FP32, tag=f"{nm}{sfx}", name=f"{nm}{sfx}")
                        nc.scalar.activation(
                            w[:cn], u[:cn], mybir.ActivationFunctionType.Relu,
                            bias=1.0, scale=-1.0,
                        )
                        wyx.append(w)
                    st = stp.tile([128, G], MM_DTYPE, tag=f"st{sfx}", name=f"st{sfx}")
                    nc.vector.tensor_tensor(
                        st[:cn, :G].rearrange("c (i j) -> c i j", i=GS),
                        wyx[0][:cn].unsqueeze(2).broadcast_to((cn, GS, GS)),
                        wyx[1][:cn].unsqueeze(1).broadcast_to((cn, GS, GS)),
                        OP.mult,
                    )
                    st_tiles[(b, ci)] = st

            for tt in range(T // T_DMA):
                _prefetch_x(1, tt)

            # bridge spins: keep the PE activity monitor fed while the last
            # Wtail copies land, so phase 1 enters at the warm clock. The
            # last two read late-setup tiles so the scheduler places them in
            # the PE gap right before phase 1.
            for s_ in range(6):
                nc.tensor.matmul(
                    spin_ps[:, :T_PS], dummy[:, :128], dummy[:], start=True, stop=True
                )


            # ---- Main loop: per batch, phase 1 = all mm1 (24 dense
            # ---- streams, ci-outer weight reuse, PSUM fully banked);
            # ---- phase 2 = all mm2. Copies drain during the phases with
            # ---- a full phase of runway, so the PE never waits on them.
            NH = T // (2 * T_PS)  # 1024-col halves per batch (4)
            # phase order: b0-mm1, b1-mm1, b0-mm2, b1-mm2 -- one continuous
            # PE stream; every PSUM->SBUF copy has a full phase of runway.
            gvs = {}
            gv_sbs = {}
            k_gv = 0
            for b in range(BL):
                for hh in range(NH):
                    gvs[(b, hh)] = psp.tile(
                        [G, 2 * T_PS], FP32, tag="pb", name=f"gv{b}_{hh}"
                    )
                for ci, (c0, cn) in enumerate(C_CHUNKS):
                    for hh in range(NH):
                        xt = x_tiles[(b, hh // 2, ci)]
                        for q in range(2):
                            f0 = (hh % 2) * 2 * T_PS + q * T_PS
                            nc.tensor.matmul(
                                gvs[(b, hh)][:, q * T_PS : (q + 1) * T_PS],
                                st_tiles[(b, ci)][:cn],
                                xt[:cn, f0 : f0 + T_PS],
                                start=(ci == 0),
                                stop=(ci == n_chunks - 1),
                                skip_group_check=True,
                            )
                for hh in range(NH):
                    gv_sb = gvp.tile(
                        [G, 2 * T_PS], MM_DTYPE,
                        tag=f"gv_sb{b}_{hh}", name=f"gv_sb{b}_{hh}",
                    )
                    if k_gv % 2 == 0:
                        nc.vector.tensor_copy(gv_sb[:], gvs[(b, hh)][:])
                    else:
                        nc.scalar.copy(gv_sb[:], gvs[(b, hh)][:])
                    k_gv += 1
                    gv_sbs[(b, hh)] = gv_sb

            k = 0
            outs = {}
            for b in range(BL):
                for mi in range(2):
                    for tt in range(2):
                        outs[(b, mi, tt)] = outp.tile(
                            [128, T_DMA], MM_DTYPE,
                            tag=f"o{mi}{tt}", name=f"o{b}_{mi}{tt}",
                        )
            # phase 2 interleaved across batches; the packed mi=2 block (8
            # streams, one copy) sits in the middle as a copy-engine
            # catch-up window. Out DMAs fire as soon as each half-tile's
            # copies land, so the store stream overlaps compute.
            for mi in (0, 1, 2):
                for b in range(BL):
                    if mi == 2:
                        # rows 256:270: all 4 t-slabs packed into ONE psum
                        # tile at partition offsets 0/32/64/96; lhsT widened
                        # to 32 rows (238:270) so every partition is written.
                        o2 = psp.tile(
                            [128, 2 * T_PS], FP32, tag="pb", name="o2"
                        )
                        for hh in range(NH):
                            p0 = 32 * hh
                            for q in range(2):
                                nc.tensor.matmul(
                                    o2[p0 : p0 + 32, q * T_PS : (q + 1) * T_PS],
                                    gw_t[:, M - 32 :],
                                    gv_sbs[(b, hh)][:G, q * T_PS : (q + 1) * T_PS],
                                    start=True,
                                    stop=True,
                                    skip_group_check=True,
                                    tile_position=(0, p0),
                                )
                        stage2 = gvp.tile(
                            [128, 2 * T_PS], MM_DTYPE,
                            tag=f"stage2_{b}", name=f"stage2_{b}",
                        )
                        if k % 2 == 0:
                            nc.vector.tensor_copy(stage2[:], o2[:])
                        else:
                            nc.scalar.copy(stage2[:], o2[:])
                        k += 1
                        for hh in range(NH):
                            p0 = 32 * hh + (M_TAIL0 - (M - 32))
                            nc.sync.dma_start(
                                out=out_ext[
                                    b, M_TAIL0:,
                                    hh * 2 * T_PS : (hh + 1) * 2 * T_PS,
                                ],
                                in_=stage2[p0 : p0 + M_TAIL, :],
                            )
                        continue
                    m0, mn = M_CHUNKS[mi]
                    for hh in range(NH):
                        o_ps = psp.tile(
                            [128, 2 * T_PS], FP32, tag="pb", name="o_ps"
                        )
                        for q in range(2):
                            nc.tensor.matmul(
                                o_ps[:mn, q * T_PS : (q + 1) * T_PS],
                                gw_t[:, m0 : m0 + mn],
                                gv_sbs[(b, hh)][:G, q * T_PS : (q + 1) * T_PS],
                                start=True,
                                stop=True,
                                skip_group_check=True,
                            )
                        dst = outs[(b, mi, hh // 2)][
                            :mn, (hh % 2) * 2 * T_PS : (hh % 2 + 1) * 2 * T_PS
                        ]
                        if k % 2 == 0:
                            nc.vector.tensor_copy(dst, o_ps[:mn])
                        else:
                            nc.scalar.copy(dst, o_ps[:mn])
                        k += 1
                        if hh % 2 == 1:
                            tt = hh // 2
                            nc.sync.dma_start(
                                out=out_ext[
                                    b, m0 : m0 + mn,
                                    tt * T_DMA : (tt + 1) * T_DMA,
                                ],
                                in_=outs[(b, mi, tt)][:mn],
                            )
    nc.compile()
    return nc


def make_in_maps(x, positions, grid_weights):
    gw_t = np.ascontiguousarray(grid_weights.T).astype(NP_MM)
    in_maps = []
    for i in range(N_CORES):
        sl = slice(i * BL, (i + 1) * BL)
        in_maps.append(
            {
                "x": np.ascontiguousarray(x[sl]).astype(NP_MM),
                "positions": np.ascontiguousarray(positions[sl]),
                "gw_t": gw_t,
            }
        )
    return in_maps


_NC_CACHE = None


def kernel(x, positions, grid_weights):
    global _NC_CACHE
    if _NC_CACHE is None:
        _NC_CACHE = build_nc()
    nc = _NC_CACHE
    in_maps = make_in_maps(x, positions, grid_weights)
    res = run_bass_kernel_spmd(nc, in_maps, core_ids=list(range(N_CORES)))
    out = np.concatenate([r["out"] for r in res.results], axis=0)
    return np.asarray(out, dtype=np.float32)


if __name__ == "__main__":
    xs = np.random.randn(B, C, T).astype(np.float32)
    ps = np.random.uniform(-1, 0.74, (B, C, 2)).astype(np.float32)
    gw = np.random.randn(M, G).astype(np.float32)
    out = kernel(xs, ps, gw)
    print(out.shape, out.dtype)


# revision 44
# speedup vs baseline: 1.1594x; 1.1594x over previous
"""AdaptiveGridMerger Trainium2 kernel.

Math: the reference scatters x[b,c,:] into a flat 8x8 grid with bilinear
(4-corner) weights from positions[b,c,:], then matmuls grid_weights
(GW [270,64]). Equivalent form used here, per batch:
  S in R[64,306] holds channel c's 4 corner weights in column c
  (row c of S.T factorizes as wy (x) wx, the bilinear hat functions),
  out[0:256]  = GW[0:256] @ (S @ x)          (two matmul passes)
  out[256:270]= (GW[256:270] @ S) @ x        (tail folded into pass 1:
     Wtail.T [306,14] is appended to S.T as extra columns, so the 14
     leftover output rows cost zero extra PE streams).
All contractions run on the TensorEngine in bf16.

Sharding: data-parallel over batch, 2 batches per core, grid_weights
replicated (pre-transposed on host to [64, 270] for the lhsT layout).

Perf structure: spin matmuls pre-ramp the PE clock (HAM gate) during
setup; weights-outer matmul ordering; x tiles fully prefetched up front
on the HW-DGE path (first tile first) so output DMAs can't head-of-line
block them; PSUM->SBUF copies are split across DVE and ACT.
"""

import numpy as np

import concourse.bass as bass
import concourse.bacc as bacc
import concourse.mybir as mybir
from concourse import tile
from concourse.bass_utils import run_bass_kernel_spmd

B, C, T = 16, 306, 4096
M, G, GS = 270, 64, 8
N_CORES = 8
BL = B // N_CORES  # batches per core

C_CHUNKS = [(0, 128), (128, 128), (256, 50)]
M_CHUNKS = [(0, 128), (128, 128), (256, 14)]
M_TAIL0, M_TAIL = 256, 14
GE = G
T_DMA = 2048
T_PS = 512
NPS = T_DMA // T_PS
N_SPIN = 13

MM_DTYPE = mybir.dt.bfloat16
NP_MM = mybir.dt.np(MM_DTYPE)

FP32 = mybir.dt.float32
OP = mybir.AluOpType


def _pos_col(b, ci):
    return 2 * (b * 2 + ci) if ci < 2 else 8 + 2 * b


def build_nc():
    nc = bacc.Bacc()
    x_ext = nc.declare_dram_parameter("x", [BL, C, T], MM_DTYPE, isOutput=False)
    pos_ext = nc.declare_dram_parameter("positions", [BL, C, 2], FP32, isOutput=False)
    gwt_ext = nc.declare_dram_parameter("gw_t", [G, M], MM_DTYPE, isOutput=False)
    out_ext = nc.declare_dram_parameter("out", [BL, M, T], MM_DTYPE, isOutput=True)

    n_chunks = len(C_CHUNKS)
    with tile.TileContext(nc) as tc:
        with (
            tc.tile_pool(name="const", bufs=1) as constp,
            tc.tile_pool(name="stp", bufs=1) as stp,
            tc.tile_pool(name="scr", bufs=1) as scr,
            tc.tile_pool(name="xp", bufs=1) as xp,
            tc.tile_pool(name="op", bufs=2) as outp,
            tc.tile_pool(name="gvp", bufs=3) as gvp,
            tc.tile_pool(name="ps", bufs=4, space=bass.MemorySpace.PSUM) as psp,
        ):
            # ---- PE clock pre-ramp while DMAs/DVE setup run.
            dummy = constp.tile([128, T_PS], MM_DTYPE, tag="dummy")
            nc.gpsimd.memset(dummy[:], 0.0)
            spin_ps = psp.tile([128, 2 * T_PS], FP32, tag="pb", name="spin_ps")
            for s in range(N_SPIN):
                nc.tensor.matmul(
                    spin_ps[:, :T_PS], dummy[:, :128], dummy[:], start=True, stop=True
                )

            # t-tile 0's x chunks first: they gate the first real matmuls
            x_tiles = {}
            for ci, (c0, cn) in enumerate(C_CHUNKS):
                xt = xp.tile(
                    [128, T_DMA], MM_DTYPE, tag=f"x0_0_{ci}", name=f"x0_0_{ci}"
                )
                nc.sync.dma_start(out=xt[:cn], in_=x_ext[0, c0 : c0 + cn, 0:T_DMA])
                x_tiles[(0, 0, ci)] = xt

            for ci, (c0, cn) in enumerate(C_CHUNKS):
                xt = xp.tile(
                    [128, T_DMA], MM_DTYPE, tag=f"x0_1_{ci}", name=f"x0_1_{ci}"
                )
                nc.sync.dma_start(
                    out=xt[:cn], in_=x_ext[0, c0 : c0 + cn, T_DMA : 2 * T_DMA]
                )
                x_tiles[(0, 1, ci)] = xt

            gw_t = constp.tile([G, M], MM_DTYPE, tag="gw_t")
            nc.sync.dma_start(out=gw_t[:], in_=gwt_ext[:])

            # iota row [0..7] (cell centers)
            io_g = constp.tile([128, GS], FP32, tag="io_g")
            nc.gpsimd.iota(
                io_g[:],
                pattern=[[1, GS]],
                base=0,
                channel_multiplier=0,
                allow_small_or_imprecise_dtypes=True,
            )
            io = constp.tile([128, GS], FP32, tag="io")
            nc.vector.tensor_copy(io[:], io_g[:])

            # ---- pos -> gp for all 6 (batch, chunk) column-pairs at once
            NCOL = 2 * BL * n_chunks
            pos_all = scr.tile([128, NCOL], FP32, tag="pos_all")
            nc.vector.memset(pos_all[:], 0.0)
            for b in range(BL):
                nc.sync.dma_start(
                    out=pos_all[:, 4 * b : 4 * b + 4].rearrange(
                        "p (ci d) -> p ci d", ci=2
                    ),
                    in_=pos_ext[b, 0:256, :].rearrange("(ci p) d -> p ci d", p=128),
                )
                nc.sync.dma_start(
                    out=pos_all[:50, 8 + 2 * b : 10 + 2 * b],
                    in_=pos_ext[b, 256:306, :],
                )

            # prefetch the rest of batch 0's x tiles (batch 1's are issued
            # after the ST build so these get the full bandwidth first)
            def _prefetch_x(b, tt):
                t0 = tt * T_DMA
                for ci, (c0, cn) in enumerate(C_CHUNKS):
                    xt = xp.tile(
                        [128, T_DMA], MM_DTYPE,
                        tag=f"x{b}_{tt}_{ci}", name=f"x{b}_{tt}_{ci}",
                    )
                    nc.sync.dma_start(
                        out=xt[:cn], in_=x_ext[b, c0 : c0 + cn, t0 : t0 + T_DMA]
                    )
                    x_tiles[(b, tt, ci)] = xt



            gp = scr.tile([128, NCOL], FP32, tag="gp")
            nc.vector.tensor_scalar(gp[:], pos_all[:], 1.0, GS / 2.0, OP.add, OP.mult)
            ngp = scr.tile([128, NCOL], FP32, tag="ngp")
            nc.vector.tensor_scalar_mul(ngp[:], gp[:], -1.0)

            # ---- ST build: st[:, :64] = wy (x) wx (hat functions)
            st_tiles = {}
            for b in range(BL):
                for ci, (c0, cn) in enumerate(C_CHUNKS):
                    sfx = f"{b}_{ci}"
                    col = _pos_col(b, ci)
                    wyx = []
                    for d, nm in ((0, "wy"), (1, "wx")):
                        cd = col + d
                        # hat on ACT: u = |io - gp|, w = relu(1 - u); frees DVE
                        u = scr.tile([128, GS], FP32, tag=f"{nm}u{sfx}", name=f"{nm}u{sfx}")
                        nc.scalar.activation(
                            u[:cn], io[:cn], mybir.ActivationFunctionType.Abs,
                            bias=ngp[:cn, cd : cd + 1], scale=1.0,
                        )
                        w = scr.tile([128, GS], FP32, tag=f"{nm}{sfx}", name=f"{nm}{sfx}")
                        nc.scalar.activation(
                            w[:cn], u[:cn], mybir.ActivationFunctionType.Relu,
                            bias=1.0, scale=-1.0,
                        )
                        wyx.append(w)
                    st = stp.tile([128, G], MM_DTYPE, tag=f"st{sfx}", name=f"st{sfx}")
                    nc.vector.tensor_tensor(
                        st[:cn, :G].rearrange("c (i j) -> c i j", i=GS),
                        wyx[0][:cn].unsqueeze(2).broadcast_to((cn, GS, GS)),
                        wyx[1][:cn].unsqueeze(1).broadcast_to((cn, GS, GS)),
                        OP.mult,
                    )
                    st_tiles[(b, ci)] = st

            for tt in range(T // T_DMA):
                _prefetch_x(1, tt)

            # bridge spins: keep the PE activity monitor fed while the last
            # Wtail copies land, so phase 1 enters at the warm clock. The
            # last two read late-setup tiles so the scheduler places them in
            # the PE gap right before phase 1.
            for s_ in range(10):
                nc.tensor.matmul(
                    spin_ps[:, :T_PS], dummy[:, :128], dummy[:], start=True, stop=True
                )


            # ---- Main loop: per batch, phase 1 = all mm1 (24 dense
            # ---- streams, ci-outer weight reuse, PSUM fully banked);
            # ---- phase 2 = all mm2. Copies drain during the phases with
            # ---- a full phase of runway, so the PE never waits on them.
            NH = T // (2 * T_PS)  # 1024-col halves per batch (4)
            # phase order: b0-mm1, b1-mm1, b0-mm2, b1-mm2 -- one continuous
            # PE stream; every PSUM->SBUF copy has a full phase of runway.
            gvs = {}
            gv_sbs = {}
            k_gv = 0
            for b in range(BL):
                for hh in range(NH):
                    gvs[(b, hh)] = psp.tile(
                        [G, 2 * T_PS], FP32, tag="pb", name=f"gv{b}_{hh}"
                    )
                for ci, (c0, cn) in enumerate(C_CHUNKS):
                    for hh in range(NH):
                        xt = x_tiles[(b, hh // 2, ci)]
                        for q in range(2):
                            f0 = (hh % 2) * 2 * T_PS + q * T_PS
                            nc.tensor.matmul(
                                gvs[(b, hh)][:, q * T_PS : (q + 1) * T_PS],
                                st_tiles[(b, ci)][:cn],
                                xt[:cn, f0 : f0 + T_PS],
                                start=(ci == 0),
                                stop=(ci == n_chunks - 1),
                                skip_group_check=True,
                            )
                for hh in range(NH):
                    gv_sb = gvp.tile(
                        [G, 2 * T_PS], MM_DTYPE,
                        tag=f"gv_sb{b}_{hh}", name=f"gv_sb{b}_{hh}",
                    )
                    if k_gv % 2 == 0:
                        nc.vector.tensor_copy(gv_sb[:], gvs[(b, hh)][:])
                    else:
                        nc.scalar.copy(gv_sb[:], gvs[(b, hh)][:])
                    k_gv += 1
                    gv_sbs[(b, hh)] = gv_sb

            k = 0
            outs = {}
            for b in range(BL):
                for mi in range(2):
                    for tt in range(2):
                        outs[(b, mi, tt)] = outp.tile(
                            [128, T_DMA], MM_DTYPE,
                            tag=f"o{mi}{tt}", name=f"o{b}_{mi}{tt}",
                        )
            # phase 2 interleaved across batches; the packed mi=2 block (8
            # streams, one copy) sits in the middle as a copy-engine
            # catch-up window. Out DMAs fire as soon as each half-tile's
            # copies land, so the store stream overlaps compute.
            for mi in (0, 1, 2):
                for b in range(BL):
                    if mi == 2:
                        # rows 256:270: all 4 t-slabs packed into ONE psum
                        # tile at partition offsets 0/32/64/96; lhsT widened
                        # to 32 rows (238:270) so every partition is written.
                        o2 = psp.tile(
                            [128, 2 * T_PS], FP32, tag="pb", name="o2"
                        )
                        for hh in range(NH):
                            p0 = 32 * hh
                            for q in range(2):
                                nc.tensor.matmul(
                                    o2[p0 : p0 + 32, q * T_PS : (q + 1) * T_PS],
                                    gw_t[:, M - 32 :],
                                    gv_sbs[(b, hh)][:G, q * T_PS : (q + 1) * T_PS],
                                    start=True,
                                    stop=True,
                                    skip_group_check=True,
                                    tile_position=(0, p0),
                                )
                        stage2 = gvp.tile(
                            [128, 2 * T_PS], MM_DTYPE,
                            tag=f"stage2_{b}", name=f"stage2_{b}",
                        )
                        if k % 2 == 0:
                            nc.vector.tensor_copy(stage2[:], o2[:])
                        else:
                            nc.scalar.copy(stage2[:], o2[:])
                        k += 1
                        for hh in range(NH):
                            p0 = 32 * hh + (M_TAIL0 - (M - 32))
                            nc.sync.dma_start(
                                out=out_ext[
                                    b, M_TAIL0:,
                                    hh * 2 * T_PS : (hh + 1) * 2 * T_PS,
                                ],
                                in_=stage2[p0 : p0 + M_TAIL, :],
                            )
                        continue
                    m0, mn = M_CHUNKS[mi]
                    for hh in range(NH):
                        o_ps = psp.tile(
                            [128, 2 * T_PS], FP32, tag="pb", name="o_ps"
                        )
                        for q in range(2):
                            nc.tensor.matmul(
                                o_ps[:mn, q * T_PS : (q + 1) * T_PS],
                                gw_t[:, m0 : m0 + mn],
                                gv_sbs[(b, hh)][:G, q * T_PS : (q + 1) * T_PS],
                                start=True,
                                stop=True,
                                skip_group_check=True,
                            )
                        dst = outs[(b, mi, hh // 2)][
                            :mn, (hh % 2) * 2 * T_PS : (hh % 2 + 1) * 2 * T_PS
                        ]
                        if k % 2 == 0:
                            nc.vector.tensor_copy(dst, o_ps[:mn])
                        else:
                            nc.scalar.copy(dst, o_ps[:mn])
                        k += 1
                        if hh % 2 == 1:
                            tt = hh // 2
                            nc.sync.dma_start(
                                out=out_ext[
                                    b, m0 : m0 + mn,
                                    tt * T_DMA : (tt + 1) * T_DMA,
                                ],
                                in_=outs[(b, mi, tt)][:mn],
                            )
    nc.compile()
    return nc


def make_in_maps(x, positions, grid_weights):
    gw_t = np.ascontiguousarray(grid_weights.T).astype(NP_MM)
    in_maps = []
    for i in range(N_CORES):
        sl = slice(i * BL, (i + 1) * BL)
        in_maps.append(
            {
                "x": np.ascontiguousarray(x[sl]).astype(NP_MM),
                "positions": np.ascontiguousarray(positions[sl]),
                "gw_t": gw_t,
            }
        )
    return in_maps


_NC_CACHE = None


def kernel(x, positions, grid_weights):
    global _NC_CACHE
    if _NC_CACHE is None:
        _NC_CACHE = build_nc()
    nc = _NC_CACHE
    in_maps = make_in_maps(x, positions, grid_weights)
    res = run_bass_kernel_spmd(nc, in_maps, core_ids=list(range(N_CORES)))
    out = np.concatenate([r["out"] for r in res.results], axis=0)
    return np.asarray(out, dtype=np.float32)


if __name__ == "__main__":
    xs = np.random.randn(B, C, T).astype(np.float32)
    ps = np.random.uniform(-1, 0.74, (B, C, 2)).astype(np.float32)
    gw = np.random.randn(M, G).astype(np.float32)
    out = kernel(xs, ps, gw)
    print(out.shape, out.dtype)
